# revision 23
# baseline (speedup 1.0000x reference)
"""CrossLingualAttention Trainium2 kernel (fp8 DoubleRow edition).

Sharding: data-parallel over batch B=8 across 8 NeuronCores, one batch
element per core; parameters replicated (gathered per-core by language
id on host). One SPMD Bass/Tile program.

Math (per batch element i, lang l = language_ids[i]):
  q = h @ Wq[l].T + bq[l];  Q = q @ wq.T + bq_  -> folded on host:
      Q = h @ Wcq + bcq,  Wcq = (wq @ Wq[l]).T, bcq = wq@bq[l]+bq_
  (same for K);  V = h @ wv.T            (MHA v-bias folded: bv flows
      linearly through attention into ctx, so bv @ G joins b_post)
  attn = softmax(Q K^T / 8) per head (no masking: reference's
      key_padding_mask bug makes the mask a no-op)
  out = (ctx @ Wo^T + bo) @ S1..S7 @ Wp^T + bp = ctx @ G + b_post
      (G, b_post are weights-only folds per distinct language)
  x = out + h;  layernorm(x) * g + b

Performance structure:
  - Q/K/V projections, attn@V and the G-apply run in fp8e4m3 with
    perf_mode=DoubleRow (K=256 per instruction: operands laid out as
    [128, 2, free] k-pair tiles).  Weights are pre-scaled x64 on host
    to sit in fp8's normal range; scale-compensation folds into the
    exp scale (1/8/64/64) and the G-apply epilogue (1/4096).
  - scores (K=64 per head) stay fp16; the two heads of a 128-row
    Q^T/K^T tile occupy partition halves 0:64 / 64:128, so their
    matmuls land in distinct PE row-groups and run concurrently
    (auto tile_position from base partitions).
  - exp is one activation per (head-pair, key-tile) over a 4-bank
    PSUM tile [128, 4, 384] -> 36 activations total, fp8 output.
  - softmax denominator rides the attnV matmul as a fused ones-column
    (V tile column 64); normalization = fused reciprocal-cast of the
    den row + gpsimd partition_broadcast + one DVE multiply into the
    fp8 ctx^T tile.
  - layernorm is applied as out = x*R - M with R = g (x) rstd and
    M = g (x) (mu*rstd) - b built by K=1/K=2 broadcast matmuls, so the
    apply is 2 tensor ops per 128-row tile.
  - PSUM budget (8 banks): tag "s" [128,4,512] (scores/stats), tag
    "l" [128,2,512] (linears/G/R_b), tag "c" [128,2,512] (attnV/M_b).
"""

import ml_dtypes
import numpy as np
from contextlib import ExitStack

import concourse.bacc as bacc
import concourse.mybir as mybir
import concourse.tile as tile
from concourse.bass_utils import run_bass_kernel_spmd

F16 = mybir.dt.float16
F32 = mybir.dt.float32
F8 = mybir.dt.float8e4
AF = mybir.ActivationFunctionType
ALU = mybir.AluOpType
DR = mybir.MatmulPerfMode.DoubleRow

NPF16 = np.float16
NPF8 = ml_dtypes.float8_e4m3fn

B, S, H, NH, NL = 8, 768, 768, 12, 5
HD = H // NH          # 64
P = 128
NT = H // P           # 6 partition tiles
NKP = NT // 2         # 3 k-pair tiles for DoubleRow
FC = 384              # free-dim chunk
NFC = S // FC         # 2
HP = 80               # padded head stride in V tiles (65 used; 16B-aligned)
WSCALE = 64.0         # fp8 pre-scale on weights
EXP_SCALE = 1.0 / (8.0 * WSCALE * WSCALE)
G_SCALE = 1.0 / (WSCALE * WSCALE)

_CACHED_NC = None
LAST_RESULTS = None


def _build_program():
    nc = bacc.Bacc(None)

    # ---- per-core DRAM inputs ----
    d_htb = nc.dram_tensor("htb8", [NKP, P, 2, S], F8, kind="ExternalInput")
    d_wq = nc.dram_tensor("wq8", [NKP, P, 2, H], F8, kind="ExternalInput")
    d_wk = nc.dram_tensor("wk8", [NKP, P, 2, H], F8, kind="ExternalInput")
    d_wv = nc.dram_tensor("wv8", [NKP, P, 2, H], F8, kind="ExternalInput")
    d_g = nc.dram_tensor("g8", [NKP, P, 2, H], F8, kind="ExternalInput")
    d_hres = nc.dram_tensor("hres16", [NKP, P, 2, S], F16, kind="ExternalInput")
    d_bias = nc.dram_tensor("bias_pack", [P, NT, 4], F32, kind="ExternalInput")
    d_out = nc.dram_tensor("out", [NT, P, S], F32, kind="ExternalOutput")

    with ExitStack() as ctx:
        tc = ctx.enter_context(tile.TileContext(nc))
        per = ctx.enter_context(tc.tile_pool(name="persist", bufs=1))
        at = ctx.enter_context(tc.tile_pool(name="apool", bufs=8))
        sm = ctx.enter_context(tc.tile_pool(name="small", bufs=1))
        dn = ctx.enter_context(tc.tile_pool(name="dnorm", bufs=3))
        ln = ctx.enter_context(tc.tile_pool(name="lnp", bufs=4))
        pS = ctx.enter_context(tc.tile_pool(name="pS", bufs=1, space="PSUM"))
        pL = ctx.enter_context(tc.tile_pool(name="pL", bufs=1, space="PSUM"))
        pC = ctx.enter_context(tc.tile_pool(name="pC", bufs=1, space="PSUM"))

        # ---- initial DMA: small tensors first, big spread over queues ----
        bias_sb = sm.tile([P, NT, 4], F32)
        nc.sync.dma_start(out=bias_sb, in_=d_bias[0:P])

        wq8 = [per.tile([P, 2, H], F8, name=f"wq8_{j}", tag=f"wq{j}")
               for j in range(NKP)]
        htb = [per.tile([P, 2, S], F8, name=f"htb{j}", tag=f"htb{j}")
               for j in range(NKP)]
        wk8 = [per.tile([P, 2, H], F8, name=f"wk8_{j}", tag=f"wk{j}")
               for j in range(NKP)]
        wv8 = [per.tile([P, 2, H], F8, name=f"wv8_{j}", tag=f"wv{j}")
               for j in range(NKP)]
        nc.sync.dma_start(out=wq8[0], in_=d_wq[0])
        nc.scalar.dma_start(out=htb[0], in_=d_htb[0])
        nc.gpsimd.dma_start(out=wq8[1], in_=d_wq[1])
        nc.sync.dma_start(out=wq8[2], in_=d_wq[2])
        nc.scalar.dma_start(out=htb[1], in_=d_htb[1])
        nc.gpsimd.dma_start(out=htb[2], in_=d_htb[2])
        nc.sync.dma_start(out=wk8[0], in_=d_wk[0])
        nc.scalar.dma_start(out=wk8[1], in_=d_wk[1])
        nc.sync.dma_start(out=wk8[2], in_=d_wk[2])
        nc.gpsimd.dma_start(out=wv8[0], in_=d_wv[0])
        nc.gpsimd.dma_start(out=wv8[1], in_=d_wv[1])
        nc.gpsimd.dma_start(out=wv8[2], in_=d_wv[2])

        # ---- small constants ----
        ones1 = sm.tile([1, P], F16)
        nc.vector.memset(ones1, 1.0)
        ones128 = sm.tile([P, 1], F16)
        nc.vector.memset(ones128, 1.0)
        eps_t = sm.tile([1, 1], F32)
        nc.vector.memset(eps_t, 1e-5)


        # ---- persistent activation tiles ----
        QT = [per.tile([P, S], F16, name=f"QT{t}", tag=f"QT{t}") for t in range(NT)]
        KT = [per.tile([P, S], F16, name=f"KT{t}", tag=f"KT{t}") for t in range(NT)]
        Vsb = [per.tile([P, 2, NH, HP], F8, name=f"V{j}", tag=f"V{j}")
               for j in range(NKP)]
        ctx8 = [per.tile([P, 2, S], F8, name=f"ctx8_{j}", tag=f"ctx{j}")
                for j in range(NKP)]
        for j in range(NKP):
            nc.gpsimd.memset(Vsb[j][:, :, :, HD:HD + 1], 1.0)

        # ---- PE warmup while DMA is in flight ----
        warm_ps = pL.tile([1, P], F32, name="warm_ps", tag="l",
                          padded_shape=[1, 1024])
        for i in range(72):
            nc.tensor.matmul(warm_ps, lhsT=ones1[:, 0:1], rhs=ones1,
                             start=(i == 0), stop=(i == 71))

        def lin_dr_mm(ps, w_tiles, m, c):
            for j in range(NKP):
                nc.tensor.matmul(
                    ps[:, c, :],
                    lhsT=w_tiles[j][:, :, m * P:(m + 1) * P],
                    rhs=htb[j][:, :, c * FC:(c + 1) * FC],
                    start=(j == 0), stop=(j == NKP - 1), perf_mode=DR)

        def lin_units(w_tiles, m, bias_col, out_t):
            """two units: (c0 matmuls), (c1 matmuls + bias/cast)"""
            ps = pL.tile([P, NFC, FC], F32, name="ps_lin", tag="l",
                         padded_shape=[P, NFC, 512])

            def u0():
                lin_dr_mm(ps, w_tiles, m, 0)

            def u1():
                lin_dr_mm(ps, w_tiles, m, 1)
                nc.vector.tensor_scalar_add(
                    out=out_t.rearrange("p (c f) -> p c f", f=FC),
                    in0=ps, scalar1=bias_sb[:, m, bias_col:bias_col + 1])
            return [u0, u1]

        def v_units(m):
            """V s-tile m (no bias; folded into b_post) -> fp8 Vsb."""
            ps = pL.tile([P, NFC, FC], F32, name="ps_v", tag="l",
                         padded_shape=[P, NFC, 512])

            def mm(c):
                for j in range(NKP):
                    nc.tensor.matmul(
                        ps[:, c, :],
                        lhsT=htb[j][:, :, m * P:(m + 1) * P],
                        rhs=wv8[j][:, :, c * FC:(c + 1) * FC],
                        start=(j == 0), stop=(j == NKP - 1), perf_mode=DR)

            def u0():
                mm(0)

            def u1():
                mm(1)
                vout = Vsb[m // 2][:, m % 2, :, 0:HD].rearrange(
                    "p (c h) d -> p c h d", c=NFC)
                nc.vector.tensor_copy(
                    out=vout, in_=ps.rearrange("p c (h d) -> p c h d", d=HD))
            return [u0, u1]

        def scores_ktc(m, kt, c, ATj):
            """paired scores for heads 2m/2m+1, key-tile kt chunk c; exp."""
            ps = pS.tile([P, 2, FC], F32, name="ps_s", tag="s", bufs=2,
                         padded_shape=[P, 2, 512])
            for h in range(2):  # adjacent emission -> row-group overlap
                p0 = h * HD
                nc.tensor.matmul(
                    ps[:, h, :],
                    lhsT=KT[m][p0:p0 + HD, kt * P:(kt + 1) * P],
                    rhs=QT[m][p0:p0 + HD, c * FC:(c + 1) * FC],
                    start=True, stop=True)
            nc.scalar.activation(
                out=ATj[:, kt % 2, :, c * FC:(c + 1) * FC],
                in_=ps, func=AF.Exp, scale=EXP_SCALE)

        def attnv_units(h, ATm, pool_):
            """ctx^T rows for head h (fp8 DoubleRow) + normalization.
            pool_ selects the PSUM slot (tag c or tag l) so the two heads
            of a pair normalize concurrently."""
            tag = "c" if pool_ is pC else "l"
            ps = pool_.tile([HD + 1, NFC, FC], F32, name=f"ps_av{h}", tag=tag,
                            padded_shape=[HD + 1, NFC, 512])

            def mm(c):
                for j in range(NKP):
                    nc.tensor.matmul(
                        ps[:, c, :],
                        lhsT=Vsb[j][:, :, h, 0:HD + 1],
                        rhs=ATm[j][:, :, h % 2, c * FC:(c + 1) * FC],
                        start=(j == 0), stop=(j == NKP - 1), perf_mode=DR)

            def u0():
                mm(0)

            def u1():
                mm(1)
                # custom-DVE reciprocal misreads at partition offset 64 on
                # HW: copy the den row to a base-0 SBUF tile first
                draw = dn.tile([1, S], F32, name=f"draw{h}", tag="draw")
                nc.vector.tensor_copy(
                    out=draw.rearrange("p (c f) -> p c f", f=FC),
                    in_=ps[HD:HD + 1, :, :])
                den = dn.tile([1, S], F32, name=f"den{h}", tag="den")
                nc.vector.reciprocal_approx_fast(out=den, in_=draw)
                rb = dn.tile([HD, S], F32, name=f"rb{h}", tag="rb")
                nc.gpsimd.partition_broadcast(rb, den)
                nc.vector.tensor_tensor(
                    out=ctx8[h // 4][(h % 2) * HD:(h % 2) * HD + HD,
                                     (h // 2) % 2, :].rearrange(
                        "p (c f) -> p c f", f=FC),
                    in0=ps[0:HD, :, :],
                    in1=rb.rearrange("p (c f) -> p c f", f=FC), op=ALU.mult)
            return [u0, u1]

        # ---- prologue: Q0/K0 ----
        for u in lin_units(wq8, 0, 0, QT[0]) + lin_units(wk8, 0, 1, KT[0]):
            u()

        # attn tiles per (head-pair m, key-pair j)
        AT = [[at.tile([P, 2, 2, S], F8, name=f"at{m}_{j}", tag="at")
               for j in range(NKP)] for m in range(NT)]

        # ---- attention groups ----
        for m in range(NT):
            units = []
            if m == 0:
                for vm in range(NT):
                    units += v_units(vm)
                units += lin_units(wq8, 1, 0, QT[1])
                units += lin_units(wk8, 1, 1, KT[1])
            else:
                if m < NT - 1:
                    units += lin_units(wq8, m + 1, 0, QT[m + 1])
                    units += lin_units(wk8, m + 1, 1, KT[m + 1])
                units += attnv_units(2 * (m - 1), AT[m - 1], pC)
                units += attnv_units(2 * (m - 1) + 1, AT[m - 1], pL)
            # spread units over the 12 (kt, c) exp slots
            nslots = NT * NFC
            plan = [[] for _ in range(nslots)]
            for i, u in enumerate(units):
                plan[(i * nslots) // len(units)].append(u)
            slot = 0
            for kt in range(NT):
                for c in range(NFC):
                    scores_ktc(m, kt, c, AT[m][kt // 2])
                    for u in plan[slot]:
                        u()
                    slot += 1
            if m == 0:
                # post-attention operands, loaded during attention
                g8, hres = [], []
                for j in range(NKP):
                    w = per.tile([P, 2, H], F8, name=f"g8_{j}", tag=f"g{j}")
                    nc.gpsimd.dma_start(out=w, in_=d_g[j])
                    g8.append(w)
                    hb = per.tile([P, 2, S], F16, name=f"hres{j}",
                                  tag=f"hres{j}")
                    nc.gpsimd.dma_start(out=hb, in_=d_hres[j])
                    hres.append(hb)
        # epilogue: last head pair (concurrent via both PSUM slots)
        for u0a, u1a, u0b, u1b in [attnv_units(2 * NT - 2, AT[NT - 1], pC)
                                   + attnv_units(2 * NT - 1, AT[NT - 1], pL)]:
            u0a(); u0b(); u1a(); u1b()

        # ---- G-apply + residual; stats matmuls trail by one tile ----
        xT = [per.tile([P, S], F16, name=f"xT{t}", tag=f"xT{t}")
              for t in range(NT)]
        xsq = [per.tile([P, S], F16, name=f"xsq{t}", tag=f"xsq{t}")
               for t in range(NT)]
        stat = pS.tile([33, NFC, FC], F32, name="stat", tag="s", bufs=2,
                       padded_shape=[33, NFC, 512])

        def stats_mm(t):
            # mean -> partition 0, sumsq -> partition 32 (col-group pair)
            for c in range(NFC):
                sl = slice(c * FC, (c + 1) * FC)
                nc.tensor.matmul(stat[0:1, c, :], lhsT=ones128,
                                 rhs=xT[t][:, sl],
                                 start=(t == 0), stop=(t == NT - 1))
                nc.tensor.matmul(stat[32:33, c, :], lhsT=ones128,
                                 rhs=xsq[t][:, sl],
                                 start=(t == 0), stop=(t == NT - 1))

        for t in range(NT):
            pool_ = pL if t % 2 == 0 else pC
            ps = pool_.tile([P, NFC, FC], F32, name="ps_g",
                            tag="l" if t % 2 == 0 else "c",
                            padded_shape=[P, NFC, 512])
            for c in range(NFC):
                for j in range(NKP):
                    nc.tensor.matmul(
                        ps[:, c, :],
                        lhsT=g8[j][:, :, t * P:(t + 1) * P],
                        rhs=ctx8[j][:, :, c * FC:(c + 1) * FC],
                        start=(j == 0), stop=(j == NKP - 1), perf_mode=DR)
            if t >= 1:
                stats_mm(t - 1)
            nc.vector.scalar_tensor_tensor(
                out=xT[t].rearrange("p (c f) -> p c f", f=FC), in0=ps,
                scalar=G_SCALE, in1=hres[t // 2][:, t % 2, :].rearrange(
                    "p (c f) -> p c f", f=FC),
                op0=ALU.mult, op1=ALU.add)
            nc.scalar.activation(out=xsq[t], in_=xT[t], func=AF.Square)
        stats_mm(NT - 1)

        # ---- layernorm row stats ----
        mu16 = sm.tile([1, S], F16)
        nc.vector.tensor_scalar_mul(
            out=mu16.rearrange("p (c f) -> p c f", f=FC),
            in0=stat[0:1, :, :], scalar1=1.0 / H)
        mu2 = sm.tile([1, S], F32)
        nc.scalar.activation(out=mu2, in_=mu16, func=AF.Square)
        var = sm.tile([1, S], F32)
        nc.vector.scalar_tensor_tensor(
            out=var.rearrange("p (c f) -> p c f", f=FC),
            in0=stat[32:33, :, :], scalar=1.0 / H,
            in1=mu2.rearrange("p (c f) -> p c f", f=FC),
            op0=ALU.mult, op1=ALU.subtract)
        sd = sm.tile([1, S], F32)
        nc.scalar.activation(out=sd, in_=var, func=AF.Sqrt,
                             bias=eps_t[0:1, :], scale=1.0)
        rstd32 = sm.tile([1, S], F32)
        nc.vector.reciprocal_approx_fast(out=rstd32, in_=sd)
        rstd16 = sm.tile([1, S], F16)
        nc.scalar.copy(out=rstd16, in_=rstd32)

        # ---- apply: out = ((x - mu_b) * rstd_b) * g(p) + b(p) ----
        mu_b = pL.tile([P, NFC, FC], F32, name="mu_b", tag="l",
                       padded_shape=[P, NFC, 512])
        rstd_b = pC.tile([P, NFC, FC], F32, name="rstd_b", tag="c",
                         padded_shape=[P, NFC, 512])
        for c in range(NFC):
            sl = slice(c * FC, (c + 1) * FC)
            nc.tensor.matmul(mu_b[:, c, :], lhsT=ones1, rhs=mu16[:, sl],
                             start=True, stop=True)
        for c in range(NFC):
            sl = slice(c * FC, (c + 1) * FC)
            nc.tensor.matmul(rstd_b[:, c, :], lhsT=ones1, rhs=rstd16[:, sl],
                             start=True, stop=True)
        rstd_sb = sm.tile([P, S], F16)
        nc.scalar.copy(out=rstd_sb.rearrange("p (c f) -> p c f", f=FC),
                       in_=rstd_b)
        for t in range(NT):
            d16 = ln.tile([P, S], F16, name="d16", tag="d16")
            nc.vector.tensor_tensor(
                out=d16.rearrange("p (c f) -> p c f", f=FC),
                in0=xT[t].rearrange("p (c f) -> p c f", f=FC),
                in1=mu_b, op=ALU.subtract)
            y16 = ln.tile([P, S], F16, name="y16", tag="y16")
            if t % 2 == 0:
                nc.vector.tensor_tensor(
                    out=y16.rearrange("p (c f) -> p c f", f=FC),
                    in0=d16.rearrange("p (c f) -> p c f", f=FC),
                    in1=rstd_b, op=ALU.mult)
            else:
                nc.gpsimd.tensor_tensor(out=y16, in0=d16, in1=rstd_sb,
                                        op=ALU.mult)
            xo = ln.tile([P, S], F32, name="xo", tag="xo")
            nc.scalar.activation(out=xo, in_=y16, func=AF.Identity,
                                 bias=bias_sb[:, t, 3:4],
                                 scale=bias_sb[:, t, 2:3])
            e = nc.sync if t % 2 == 0 else nc.gpsimd
            e.dma_start(out=d_out[t], in_=xo)

    nc.finalize()
    return nc


def _q8(x):
    return np.clip(np.asarray(x, np.float32), -240.0, 240.0).astype(NPF8)


def _pair_tiles(a):
    """[768, N] -> [3, 128, 2, N] k-pair layout (row k = j*256 + s*128 + p)."""
    n = a.shape[1]
    return np.ascontiguousarray(
        a.reshape(NKP, 2, P, n).transpose(0, 2, 1, 3))


def _prep_inputs(hidden_states, language_ids, Wq_lang, bq_lang, Wk_lang,
                 bk_lang, in_proj_w, in_proj_b, out_proj_w, out_proj_b,
                 align, proj_w, proj_b, ln_g, ln_b):
    f = np.float32
    hs = np.asarray(hidden_states, f)
    lang = np.asarray(language_ids).astype(np.int64)
    Wq_lang = np.asarray(Wq_lang, f)
    bq_lang = np.asarray(bq_lang, f)
    Wk_lang = np.asarray(Wk_lang, f)
    bk_lang = np.asarray(bk_lang, f)
    in_proj_w = np.asarray(in_proj_w, f)
    in_proj_b = np.asarray(in_proj_b, f)
    out_proj_w = np.asarray(out_proj_w, f)
    out_proj_b = np.asarray(out_proj_b, f)
    align = np.asarray(align, f)
    proj_w = np.asarray(proj_w, f)
    proj_b = np.asarray(proj_b, f)
    ln_g = np.asarray(ln_g, f)
    ln_b = np.asarray(ln_b, f)

    wq, wk, wv = in_proj_w[:H], in_proj_w[H:2 * H], in_proj_w[2 * H:]
    bq_, bk_, bv_ = in_proj_b[:H], in_proj_b[H:2 * H], in_proj_b[2 * H:]
    projT = np.ascontiguousarray(proj_w.T)
    wv8 = _pair_tiles(_q8(wv.T * WSCALE))
    identity = np.eye(H, dtype=f)

    langs_present = sorted(set(lang.tolist()))
    wcq8, wck8, bcq, bck = {}, {}, {}, {}
    for l in langs_present:
        wcq8[l] = _pair_tiles(_q8((wq @ Wq_lang[l]).T * WSCALE))
        wck8[l] = _pair_tiles(_q8((wk @ Wk_lang[l]).T * WSCALE))
        bcq[l] = (wq @ bq_lang[l] + bq_) * WSCALE
        bck[l] = (wk @ bk_lang[l] + bk_) * WSCALE

    # weights-only folds per language: G = Wo^T M Wp^T; bias chain picks up
    # the MHA value bias exactly (ctx includes +bv -> +bv@G on the output)
    G8_by_lang, bpost_by_lang = {}, {}
    for l in langs_present:
        M = identity
        for j in range(B):
            lj = int(lang[j])
            if lj != l:
                M = M @ align[l, lj]
        G = out_proj_w.T @ M @ projT
        G8_by_lang[l] = _pair_tiles(_q8(G * WSCALE))
        bpost_by_lang[l] = out_proj_b @ M @ projT + proj_b + bv_ @ G

    in_maps = []
    for i in range(B):
        li = int(lang[i])
        hT = np.ascontiguousarray(hs[i].T)
        bias_pack = np.stack(
            [bcq[li].reshape(NT, P), bck[li].reshape(NT, P),
             ln_g.reshape(NT, P), ln_b.reshape(NT, P)],
            axis=-1).transpose(1, 0, 2)
        in_maps.append({
            "htb8": _pair_tiles(_q8(hT)),
            "wq8": wcq8[li],
            "wk8": wck8[li],
            "wv8": wv8,
            "g8": G8_by_lang[li],
            "hres16": _pair_tiles(
                (hT + bpost_by_lang[li][:, None]).astype(NPF16)),
            "bias_pack": np.ascontiguousarray(bias_pack.astype(f)),
        })
    return in_maps


def kernel(hidden_states, language_ids, attention_mask, Wq_lang, bq_lang,
           Wk_lang, bk_lang, in_proj_w, in_proj_b, out_proj_w, out_proj_b,
           align, proj_w, proj_b, ln_g, ln_b):
    global _CACHED_NC, LAST_RESULTS
    if _CACHED_NC is None:
        _CACHED_NC = _build_program()
    in_maps = _prep_inputs(hidden_states, language_ids, Wq_lang, bq_lang,
                           Wk_lang, bk_lang, in_proj_w, in_proj_b,
                           out_proj_w, out_proj_b, align, proj_w, proj_b,
                           ln_g, ln_b)
    res = run_bass_kernel_spmd(_CACHED_NC, in_maps, core_ids=list(range(B)))
    LAST_RESULTS = res
    return np.stack(
        [np.ascontiguousarray(res.results[i]["out"].reshape(H, S).T)
         for i in range(B)], axis=0)
